# revision 1
# baseline (speedup 1.0000x reference)
"""Trainium2 Bass kernel for nn_BasicBlock (WeightNet/CondConv-style block).

Data parallel over batch: 32 samples -> 8 cores x 4 samples.
Per core, per sample:
  gap   = mean(x, HW) @ reduce_w.T + reduce_b                  (PE + DVE)
  a_wn  = sigmoid(gap @ fc1_w.T + fc1_b)                       (PE + ACT)
  W_wn  = einsum('gi,goi->go', a, w2) per-sample 3x3 kernels   (DVE)
  out   = relu(bn1(conv(x, W1)))                               (PE + ACT)
  out   = bn2(conv(out, W2)) + x; relu                         (PE + ACT + DVE)
Conv implemented as 9 shifted bf16 matmuls accumulating in PSUM, padded
58x58 image layout in SBUF. Static weights host-pre-packed (layout only).
"""

import sys

sys.path.insert(0, "/opt/trn_rl_repo")

import numpy as np
import ml_dtypes

import concourse.bass as bass
import concourse.tile as tile
from concourse import bacc, mybir
from concourse import bass_utils

F32 = mybir.dt.float32
BF16 = mybir.dt.bfloat16
AF = mybir.ActivationFunctionType

B, C, H, W = 32, 256, 56, 56
NCORES = 8
BL = B // NCORES          # samples per core
HP, WP = H + 2, W + 2     # padded 58x58
NPIX = H * W              # 3136
NPPAD = HP * WP           # 3364
NT = 7                    # h-tiles of 8 rows
TROWS = 8
NFREE = TROWS * W         # 448 columns per matmul
EPS = 1e-5


def build_program():
    nc = bacc.Bacc("TRN2", target_bir_lowering=False, debug=False,
                   num_devices=NCORES)

    x4 = nc.dram_tensor("x4", [BL, C, H, W], F32, kind="ExternalInput").ap()
    x4b = nc.dram_tensor("x4b", [BL, C, H, W], BF16, kind="ExternalInput").ap()
    out4 = nc.dram_tensor("out4", [BL, C, H, W], F32, kind="ExternalOutput").ap()
    rwT = nc.dram_tensor("rwT", [2, 128, 16], F32, kind="ExternalInput").ap()
    rb = nc.dram_tensor("rb", [16, 1], F32, kind="ExternalInput").ap()
    fc1wT = [nc.dram_tensor(f"fc1wT{n}", [16, 4096], BF16, kind="ExternalInput").ap()
             for n in (1, 2)]
    fc1b = [nc.dram_tensor(f"fc1b{n}", [128, 32], F32, kind="ExternalInput").ap()
            for n in (1, 2)]
    w2p = [nc.dram_tensor(f"w2p{n}", [2, 128, 4 * 9 * 256], BF16,
                          kind="ExternalInput").ap() for n in (1, 2)]
    bns = [nc.dram_tensor(f"bns{n}", [2, 128, 1], F32, kind="ExternalInput").ap()
           for n in (1, 2)]
    bnb = [nc.dram_tensor(f"bnb{n}", [2, 128, 1], F32, kind="ExternalInput").ap()
           for n in (1, 2)]

    with tile.TileContext(nc) as tc:
        build_body(tc, x4, x4b, out4, rwT, rb, fc1wT, fc1b, w2p, bns, bnb)

    nc.compile()
    return nc


def build_body(tc, x4, x4b, out4, rwT, rb, fc1wT, fc1b, w2p, bns, bnb):
    nc = tc.nc
    from contextlib import ExitStack
    ctx = ExitStack()

    cpool = ctx.enter_context(tc.tile_pool(name="consts", bufs=1))
    xpad_p = ctx.enter_context(tc.tile_pool(name="xpad", bufs=4))
    o1pad_p = ctx.enter_context(tc.tile_pool(name="o1pad", bufs=4))
    aexp_p = ctx.enter_context(tc.tile_pool(name="aexp", bufs=2))
    wgen_p = ctx.enter_context(tc.tile_pool(name="wgen", bufs=6))
    wtmp_p = ctx.enter_context(tc.tile_pool(name="wtmp", bufs=2))
    small_p = ctx.enter_context(tc.tile_pool(name="small", bufs=2))
    stage_p = ctx.enter_context(tc.tile_pool(name="stage", bufs=2))
    avlin_p = ctx.enter_context(tc.tile_pool(name="avlinp", bufs=1))
    xstage_p = ctx.enter_context(tc.tile_pool(name="xstage", bufs=1))
    psum_p = ctx.enter_context(tc.tile_pool(name="psum", bufs=5, space="PSUM"))
    psmall_p = ctx.enter_context(tc.tile_pool(name="psmall", bufs=1, space="PSUM"))
    dram_p = ctx.enter_context(tc.tile_pool(name="dscratch", bufs=2, space="DRAM"))

    # sample-0 chunk-0 staging load first: it heads the critical chain
    xs_pre = xstage_p.tile([128, NPIX], BF16, tag="xstage")
    nc.sync.dma_start(xs_pre[:],
                      x4b[0, 0:128].rearrange("c h w -> c (h w)"))

    # ---- resident constants (w2sb DMAs deferred for startup overlap) ----
    w2sb = []   # [wn][chunk][i] -> [128, 2304] bf16 (k*256+co)
    for n in range(2):
        per = []
        for c in range(2):
            blocks = []
            for i in range(4):
                w2t = cpool.tile([128, 2304], BF16, tag=f"w2sb{n}{c}{i}")
                blocks.append(w2t)
            per.append(blocks)
        w2sb.append(per)

    def load_w2sb(n):
        for c in range(2):
            for i in range(4):
                nc.sync.dma_start(w2sb[n][c][i][:],
                                  w2p[n][c][:, 2304 * i:2304 * (i + 1)])
    rwT_sb = []
    for c in range(2):
        t = cpool.tile([128, 16], F32, tag=f"rwT{c}")
        nc.sync.dma_start(t[:], rwT[c])
        rwT_sb.append(t)
    rb_sb = cpool.tile([16, 1], F32, tag="rb")
    nc.sync.dma_start(rb_sb[:], rb)
    fc1wT_sb, fc1b_sb, bns_sb, bnb_sb = [], [], [], []
    for n in range(2):
        t = cpool.tile([16, 4096], BF16, tag=f"fc1wT{n}")
        if n == 0:
            nc.sync.dma_start(t[:], fc1wT[n])
        fc1wT_sb.append(t)
        t = cpool.tile([128, 32], F32, tag=f"fc1b{n}")
        if n == 0:
            nc.sync.dma_start(t[:], fc1b[n])
        fc1b_sb.append(t)
        ts, tb = [], []
        for c in range(2):
            a = cpool.tile([128, 1], F32, tag=f"bns{n}{c}")
            ts.append(a)
            a = cpool.tile([128, 1], F32, tag=f"bnb{n}{c}")
            tb.append(a)
        bns_sb.append(ts)
        bnb_sb.append(tb)

    def load_deferred_consts():
        nc.sync.dma_start(fc1wT_sb[1][:], fc1wT[1])
        nc.sync.dma_start(fc1b_sb[1][:], fc1b[1])
        for n in range(2):
            for c in range(2):
                nc.sync.dma_start(bns_sb[n][c][:], bns[n][c])
                nc.sync.dma_start(bnb_sb[n][c][:], bnb[n][c])
    gap16 = cpool.tile([16, BL], BF16, tag="gap16")
    ones_sb = cpool.tile([1, 64], BF16, tag="ones")
    nc.gpsimd.memset(ones_sb[:], 1.0)

    def border_memset(t):
        r = t[:].rearrange("p (h w) -> p h w", h=HP)
        nc.gpsimd.memset(r[:, 0, :], 0.0)
        nc.gpsimd.memset(r[:, HP - 1, :], 0.0)
        nc.gpsimd.memset(r[:, 1:HP - 1, 0:1], 0.0)
        nc.gpsimd.memset(r[:, 1:HP - 1, WP - 1:WP], 0.0)

    def gen_weights_a(wn, s):
        """sigmoid(fc1(gap)) -> partition-broadcast coefficient tiles."""
        aps = psmall_p.tile([128, 32], F32, tag="avec_ps")
        for j in range(32):
            nc.tensor.matmul(aps[:, j:j + 1],
                             fc1wT_sb[wn][:, 128 * j:128 * (j + 1)],
                             gap16[:, s:s + 1],
                             start=True, stop=True)
        avt = small_p.tile([128, 32], F32, tag="avtmp")
        nc.vector.tensor_add(avt[:], aps[:], fc1b_sb[wn][:])
        avec = small_p.tile([128, 32], BF16, tag="avec")
        nc.scalar.activation(avec[:], avt[:], AF.Sigmoid)
        avd = dram_p.tile([4096], BF16, tag="avd")
        nc.sync.dma_start(avd[:].rearrange("(j p) -> p j", p=128), avec[:])
        avlin = avlin_p.tile([1, 4096], BF16, tag="avlin")
        nc.sync.dma_start(avlin[:], avd[:].unsqueeze(0))
        avr = avlin[:].rearrange("o (co r) -> o co r", r=16)
        aexp = []
        for c in range(2):
            t = aexp_p.tile([128, 4 * 256], BF16, tag=f"aexp{c}")
            for half in range(2):
                aps2 = psmall_p.tile([128, 2 * 256], F32, tag="aexp_ps")
                for h in range(2):
                    for ii in range(2):
                        i = 2 * half + ii
                        m = 4 * (2 * c + h) + i
                        rhs = avr[:, :, m:m + 1].rearrange("o co r -> o (co r)")
                        nc.tensor.matmul(
                            aps2[64 * h:64 * (h + 1), 256 * ii:256 * (ii + 1)],
                            ones_sb[:], rhs, start=True, stop=True)
                nc.scalar.copy(t[:, 512 * half:512 * (half + 1)], aps2[:])
            aexp.append(t)
        return aexp

    def gen_weights_b(wn, aexp):
        wt = []
        for c in range(2):
            t = wgen_p.tile([128, 9 * 256], BF16, tag="wgen")

            def abid(i):
                return (aexp[c][:, 256 * i:256 * (i + 1)].unsqueeze(1)
                        .broadcast_to([128, 9, 256]))

            def k3(ap2d, lo):
                return ap2d[:, lo:lo + 2304].rearrange(
                    "p (k co) -> p k co", k=9)

            nc.vector.tensor_mul(k3(t[:], 0), k3(w2sb[wn][c][0][:], 0), abid(0))
            for i in range(1, 4):
                tmp = wtmp_p.tile([128, 9 * 256], BF16, tag="wtmp")
                nc.vector.tensor_mul(
                    k3(tmp[:], 0), k3(w2sb[wn][c][i][:], 0), abid(i))
                nc.vector.tensor_add(t[:], t[:], tmp[:])
            wt.append(t)
        return wt

    def gen_weights(wn, s):
        return gen_weights_b(wn, gen_weights_a(wn, s))

    def conv(wt, src_pads, sink):
        """9-offset shifted matmul conv; sink(cc, t, psum_tile) evacuates."""
        for cc in range(2):
            for t in range(NT):
                ps = psum_p.tile([128, NFREE], F32, tag="cps")
                first = True
                for c in range(2):
                    xr = src_pads[c][:].rearrange("p (h w) -> p h w", h=HP)
                    for kh in range(3):
                        for kw in range(3):
                            k = 3 * kh + kw
                            nc.tensor.matmul(
                                ps[:],
                                wt[c][:, 256 * k + 128 * cc:
                                      256 * k + 128 * cc + 128],
                                xr[:, TROWS * t + kh:TROWS * t + kh + TROWS,
                                   kw:kw + W],
                                start=first, stop=(c == 1 and k == 8))
                            first = False
                sink(cc, t, ps)

    def load_x_gap(s, pre=None):
        xpad = []
        gsum = []
        for c in range(2):
            if c == 0 and pre is not None:
                xs = pre
            else:
                xs = xstage_p.tile([128, NPIX], BF16, tag="xstage")
                nc.sync.dma_start(
                    xs[:],
                    x4b[s, 128 * c:128 * (c + 1)].rearrange("c h w -> c (h w)"))
            g = small_p.tile([128, 1], F32, tag="gsum")
            xp = xpad_p.tile([128, NPPAD], BF16, tag="xpad")
            border_memset(xp)
            xpr = xp[:].rearrange("p (h w) -> p h w", h=HP)
            nc.scalar.activation(xpr[:, 1:1 + H, 1:1 + W],
                                 xs[:].rearrange("p (h w) -> p h w", h=H),
                                 AF.Copy, accum_out=g[:])
            xpad.append(xp)
            gsum.append(g)
        gps = psmall_p.tile([16, 1], F32, tag="gap_ps")
        for c in range(2):
            nc.tensor.matmul(gps[:], rwT_sb[c][:], gsum[c][:],
                             start=(c == 0), stop=(c == 1))
        nc.scalar.activation(gap16[:, s:s + 1], gps[:], AF.Identity,
                             bias=rb_sb[:], scale=1.0)
        return xpad

    xpad = load_x_gap(0, pre=xs_pre)
    ax0 = gen_weights_a(0, 0)
    load_w2sb(0)
    w1 = gen_weights_b(0, ax0)
    load_deferred_consts()
    load_w2sb(1)
    xpad_next = None
    w1_next = None

    for s in range(BL):
        w2 = gen_weights(1, s)
        if s + 1 < BL:
            xpad_next = load_x_gap(s + 1)
            w1_next = gen_weights(0, s + 1)

        # ---- conv1 + bn1 + relu -> o1pad (bf16, padded) ----
        o1pad = []
        for c in range(2):
            op = o1pad_p.tile([128, NPPAD], BF16, tag="o1pad")
            border_memset(op)
            o1pad.append(op)

        def sink1(cc, t, ps):
            opr = o1pad[cc][:].rearrange("p (h w) -> p h w", h=HP)
            nc.scalar.activation(
                opr[:, TROWS * t + 1:TROWS * t + 1 + TROWS, 1:1 + W],
                ps[:].rearrange("p (h w) -> p h w", h=TROWS),
                AF.Relu, bias=bnb_sb[0][cc][:], scale=bns_sb[0][cc][:])

        conv(w1, xpad, sink1)

        # ---- conv2 + bn2 + residual + relu -> out ----
        def sink2(cc, t, ps):
            t2 = stage_p.tile([128, NFREE], F32, tag="t2")
            nc.scalar.activation(t2[:], ps[:], AF.Identity,
                                 bias=bnb_sb[1][cc][:], scale=bns_sb[1][cc][:])
            xres = stage_p.tile([128, NFREE], F32, tag="xres")
            xflat = x4[s, 128 * cc:128 * (cc + 1)].rearrange("c h w -> c (h w)")
            nc.sync.dma_start(xres[:], xflat[:, NFREE * t:NFREE * (t + 1)])
            nc.vector.tensor_add(t2[:], t2[:], xres[:])
            nc.vector.tensor_scalar_max(t2[:], t2[:], 0.0)
            oflat = out4[s, 128 * cc:128 * (cc + 1)].rearrange("c h w -> c (h w)")
            nc.sync.dma_start(oflat[:, NFREE * t:NFREE * (t + 1)], t2[:])

        conv(w2, o1pad, sink2)
        xpad = xpad_next
        w1 = w1_next

    ctx.close()


_NC_CACHE = {}


def get_program():
    if "nc" not in _NC_CACHE:
        _NC_CACHE["nc"] = build_program()
    return _NC_CACHE["nc"]


def prep_inputs(inputs):
    x = np.asarray(inputs["x"], np.float32)
    f32 = lambda a: np.ascontiguousarray(np.asarray(a, np.float32))
    bf = lambda a: np.ascontiguousarray(
        np.asarray(a, np.float32).astype(ml_dtypes.bfloat16))

    def pack_w2(fc2_w):
        w2_ = np.asarray(fc2_w, np.float32).reshape(256, 4, 64, 9, 4)
        w2h = w2_.transpose(4, 3, 1, 2, 0).reshape(4, 9, 256, 256)
        return bf(w2h.transpose(2, 0, 1, 3).reshape(2, 128, 4 * 9 * 256))

    def bn_fold(g, b, m, v):
        sc = np.asarray(g, np.float32) / np.sqrt(np.asarray(v, np.float32) + EPS)
        bia = np.asarray(b, np.float32) - np.asarray(m, np.float32) * sc
        return f32(sc.reshape(2, 128, 1)), f32(bia.reshape(2, 128, 1))

    base = {
        "rwT": f32((np.asarray(inputs["reduce_w"], np.float32).T / NPIX)
                   .reshape(2, 128, 16)),
        "rb": f32(np.asarray(inputs["reduce_b"]).reshape(16, 1)),
        "fc1wT1": bf(np.asarray(inputs["w1_fc1_w"]).T),
        "fc1wT2": bf(np.asarray(inputs["w2_fc1_w"]).T),
        "fc1b1": f32(np.asarray(inputs["w1_fc1_b"]).reshape(32, 128).T),
        "fc1b2": f32(np.asarray(inputs["w2_fc1_b"]).reshape(32, 128).T),
        "w2p1": pack_w2(inputs["w1_fc2_w"]),
        "w2p2": pack_w2(inputs["w2_fc2_w"]),
    }
    base["bns1"], base["bnb1"] = bn_fold(inputs["bn1_g"], inputs["bn1_b"],
                                         inputs["bn1_m"], inputs["bn1_v"])
    base["bns2"], base["bnb2"] = bn_fold(inputs["bn2_g"], inputs["bn2_b"],
                                         inputs["bn2_m"], inputs["bn2_v"])
    xb = x.astype(ml_dtypes.bfloat16)
    in_maps = []
    for i in range(NCORES):
        m = dict(base)
        m["x4"] = np.ascontiguousarray(x[i * BL:(i + 1) * BL])
        m["x4b"] = np.ascontiguousarray(xb[i * BL:(i + 1) * BL])
        in_maps.append(m)
    return in_maps


def kernel(**inputs):
    in_maps = prep_inputs(inputs)
    nc = get_program()
    res = bass_utils.run_bass_kernel_spmd(nc, in_maps,
                                          core_ids=list(range(NCORES)))
    out = np.concatenate([r["out4"] for r in res.results], axis=0)
    return out.astype(np.float32)



# revision 35
# speedup vs baseline: 1.1150x; 1.1150x over previous
"""Trainium2 Bass kernel for nn_BasicBlock (WeightNet/CondConv-style block).

Data parallel over batch: 32 samples -> 8 cores x 4 samples.
Conv implemented as 1D Winograd F(2,3) along H: per output-row-pair ty,
V_f = B^T over 4 input rows (DVE adds), 4 freq GEMMs over (kw, ic) on PE
(6 matmuls x 224 rows per psum vs direct conv's 18 x 448 -> 1.5x fewer
PE rows), A^T recombine + bn/relu on Pool/ACT.  Per-sample 3x3 weights
are generated on-device: gap -> fc1 -> sigmoid -> PE transpose (no DRAM
roundtrip) -> ones-broadcast matmuls -> DVE combine with a streamed
basis (kh-split, kh1 pre-halved so the Winograd G-transform is 3 fused
ops).  Residual is re-read from the bf16 padded input (no f32 x copy).
"""

import sys

sys.path.insert(0, "/opt/trn_rl_repo")

import numpy as np
import ml_dtypes

import concourse.bass as bass
import concourse.tile as tile
from concourse import bacc, mybir
from concourse import bass_utils
from concourse import masks

F32 = mybir.dt.float32
BF16 = mybir.dt.bfloat16
AF = mybir.ActivationFunctionType
OP = mybir.AluOpType

B, C, H, W = 32, 256, 56, 56
NCORES = 8
BL = B // NCORES          # samples per core
HP, WP = H + 2, W + 2     # padded 58x58
NPIX = H * W              # 3136
NPPAD = HP * WP           # 3364
NTY = 28                  # output row pairs
NSTR = 7                  # strips of 4 ty (8 output rows)
TYS = 4
NFREE = TYS * W           # 224 per freq psum
VFREE = 4 * NTY * WP      # 6496  (f, ty, px)
VSFREE = 4 * TYS * WP     # 928 per strip
EPS = 1e-5


def build_program():
    nc = bacc.Bacc("TRN2", target_bir_lowering=False, debug=False,
                   num_devices=NCORES)

    x4b = nc.dram_tensor("x4b", [BL, C, H, W], BF16, kind="ExternalInput").ap()
    out4 = nc.dram_tensor("out4", [BL, C, H, W], F32, kind="ExternalOutput").ap()
    rwT = nc.dram_tensor("rwT", [2, 128, 16], F32, kind="ExternalInput").ap()
    rb = nc.dram_tensor("rb", [16, 1], F32, kind="ExternalInput").ap()
    fc1wT = [nc.dram_tensor(f"fc1wT{n}", [16, 4096], BF16,
                            kind="ExternalInput").ap() for n in (1, 2)]
    fc1b = [nc.dram_tensor(f"fc1b{n}", [128, 32], F32,
                           kind="ExternalInput").ap() for n in (1, 2)]
    w2p = [nc.dram_tensor(f"w2p{n}", [2, 4, 128, 2304], BF16,
                          kind="ExternalInput").ap() for n in (1, 2)]
    bns = [nc.dram_tensor(f"bns{n}", [2, 128, 1], F32,
                          kind="ExternalInput").ap() for n in (1, 2)]
    bnb = [nc.dram_tensor(f"bnb{n}", [2, 128, 1], F32,
                          kind="ExternalInput").ap() for n in (1, 2)]
    rdg2 = nc.dram_tensor("rdg2", [2, 2, 128, 128], BF16, kind="ExternalInput").ap()

    with tile.TileContext(nc) as tc:
        build_body(tc, x4b, out4, rwT, rb, fc1wT, fc1b, w2p, bns, bnb, rdg2)

    nc.compile()
    return nc


def build_body(tc, x4b, out4, rwT, rb, fc1wT, fc1b, w2p, bns, bnb, rdg2):
    nc = tc.nc
    from contextlib import ExitStack
    ctx = ExitStack()

    cpool = ctx.enter_context(tc.tile_pool(name="consts", bufs=1))
    bsb_p = ctx.enter_context(tc.tile_pool(name="bsb", bufs=4))
    xpad_p = ctx.enter_context(tc.tile_pool(name="xpad", bufs=4))
    xstage_p = ctx.enter_context(tc.tile_pool(name="xstage", bufs=2))
    av_p = ctx.enter_context(tc.tile_pool(name="av", bufs=1))
    o1pad_p = ctx.enter_context(tc.tile_pool(name="o1pad", bufs=2))
    v1_p = ctx.enter_context(tc.tile_pool(name="v1", bufs=2))
    v2_p = ctx.enter_context(tc.tile_pool(name="v2", bufs=14))
    wt_p = ctx.enter_context(tc.tile_pool(name="wt", bufs=1))
    wtmp_p = ctx.enter_context(tc.tile_pool(name="wtmp", bufs=1))
    aexp_p = ctx.enter_context(tc.tile_pool(name="aexp", bufs=1))
    s1tmp_p = ctx.enter_context(tc.tile_pool(name="s1tmp", bufs=4))
    s2tmp_p = ctx.enter_context(tc.tile_pool(name="s2tmp", bufs=4))
    stage_p = ctx.enter_context(tc.tile_pool(name="stage", bufs=2))
    small_p = ctx.enter_context(tc.tile_pool(name="small", bufs=2))
    cps_p = ctx.enter_context(tc.tile_pool(name="cps", bufs=5, space="PSUM"))
    psmall_p = ctx.enter_context(tc.tile_pool(name="psmall", bufs=1,
                                              space="PSUM"))
    aps2_p = ctx.enter_context(tc.tile_pool(name="aps2", bufs=1, space="PSUM"))
    dram_p = ctx.enter_context(tc.tile_pool(name="dscratch", bufs=2,
                                            space="DRAM"))

    # ---- x(0) chunk 0 staging heads the critical chain ----
    xstage = {}

    def load_xstage(s, c):
        xs = xstage_p.tile([128, NPIX], BF16, tag="xstage")
        nc.sync.dma_start(xs[:],
                          x4b[s, 128 * c:128 * (c + 1)]
                          .rearrange("c h w -> c (h w)"))
        xstage[(s, c)] = xs

    dummy = cpool.tile([1, 2], F32, tag="dummy")
    nc.gpsimd.memset(dummy[:], 0.0)
    for fset in (AF.Copy, AF.Sigmoid, AF.Relu, AF.Identity):
        nc.scalar.activation(dummy[:], dummy[:], fset)

    load_xstage(0, 0)
    load_xstage(0, 1)

    # ---- resident constants ----
    rwT_sb = []
    for c in range(2):
        t = cpool.tile([128, 16], F32, tag=f"rwT{c}")
        nc.sync.dma_start(t[:], rwT[c])
        rwT_sb.append(t)
    rb_sb = cpool.tile([16, 1], F32, tag="rb")
    nc.sync.dma_start(rb_sb[:], rb)
    fc1wT_sb, fc1b_sb, bns_sb, bnb_sb = [], [], [], []
    for n in range(2):
        t = cpool.tile([16, 4096], BF16, tag=f"fc1wT{n}")
        nc.sync.dma_start(t[:], fc1wT[n])
        fc1wT_sb.append(t)
        t = cpool.tile([128, 32], F32, tag=f"fc1b{n}")
        nc.sync.dma_start(t[:], fc1b[n])
        fc1b_sb.append(t)
        ts, tb = [], []
        for c in range(2):
            a = cpool.tile([128, 1], F32, tag=f"bns{n}{c}")
            nc.sync.dma_start(a[:], bns[n][c])
            ts.append(a)
            a = cpool.tile([128, 1], F32, tag=f"bnb{n}{c}")
            nc.sync.dma_start(a[:], bnb[n][c])
            tb.append(a)
        bns_sb.append(ts)
        bnb_sb.append(tb)
    gap16 = cpool.tile([16, BL], BF16, tag="gap16")
    ones_sb = cpool.tile([1, 64], BF16, tag="ones")
    nc.gpsimd.memset(ones_sb[:], 1.0)
    ident = cpool.tile([128, 128], BF16, tag="ident")
    masks.make_identity(nc, ident[:])
    rdg_sb = []
    for c in range(2):
        pair = []
        for sgn in range(2):
            t = cpool.tile([128, 128], BF16, tag=f"rdg{c}{sgn}")
            nc.sync.dma_start(t[:], rdg2[c, sgn])
            pair.append(t)
        rdg_sb.append(pair)

    # ---- streamed basis: single rotating tag, prefetch one (c)-group ----
    basis = {}   # (wn, c, i) -> tile

    def _basis_cycle():
        # use order: w1(prologue), then per iter s: w2, w1(next)
        for c in range(2):
            for i in range(4):
                yield (0, c, i)
        for s in range(BL):
            for c in range(2):
                for i in range(4):
                    yield (1, c, i)
            if s + 1 < BL:
                for c in range(2):
                    for i in range(4):
                        yield (0, c, i)

    basis_gen = _basis_cycle()

    def load_next_basis(n):
        for _ in range(n):
            key = next(basis_gen, None)
            if key is None:
                return
            wn, c, i = key
            t = bsb_p.tile([128, 2304], BF16, tag="bs")
            nc.sync.dma_start(t[:], w2p[wn][c, i])
            basis[key] = t

    def border_memset(t):
        r = t[:].rearrange("p (h w) -> p h w", h=HP)
        nc.gpsimd.memset(r[:, 0, :], 0.0)
        nc.gpsimd.memset(r[:, HP - 1, :], 0.0)
        nc.gpsimd.memset(r[:, 1:HP - 1, 0:1], 0.0)
        nc.gpsimd.memset(r[:, 1:HP - 1, WP - 1:WP], 0.0)

    xpad = {}    # s -> [2 tiles]
    aps_t = {}   # s -> merged gap/fc1 psum tile

    def padcopy_gap(s):
        """xstage(s) -> xpad(s) with gap accumulation + gap16[:, s]."""
        gsum = []
        tiles = []
        for c in range(2):
            xs = xstage.pop((s, c))
            g = small_p.tile([128, 1], F32, tag="gsum")
            xp = xpad_p.tile([128, NPPAD], BF16, tag="xpad")
            border_memset(xp)
            xpr = xp[:].rearrange("p (h w) -> p h w", h=HP)
            nc.scalar.activation(xpr[:, 1:1 + H, 1:1 + W],
                                 xs[:].rearrange("p (h w) -> p h w", h=H),
                                 AF.Copy, accum_out=g[:])
            tiles.append(xp)
            gsum.append(g)
        xpad[s] = tiles
        apst = psmall_p.tile([128, 33], F32, tag="aps")
        aps_t[s] = apst
        gps = apst[0:16, 32:33]
        for c in range(2):
            nc.tensor.matmul(gps, rwT_sb[c][:], gsum[c][:],
                             start=(c == 0), stop=(c == 1))
        nc.scalar.activation(gap16[:, s:s + 1], gps, AF.Identity,
                             bias=rb_sb[:], scale=1.0)

    # ---- weight-gen front half: fc1 -> sigmoid -> PE transpose ----
    avvec = {}   # (wn, q) -> [1, 128] tile at base partition 0

    def front_a(wn, s, pst2):
        aps = aps_t[s][:, 0:32]
        for j in range(32):
            nc.tensor.matmul(aps[:, j:j + 1],
                             fc1wT_sb[wn][:, 128 * j:128 * (j + 1)],
                             gap16[:, s:s + 1], start=True, stop=True)
        avt = small_p.tile([128, 32], F32, tag="avt")
        nc.vector.tensor_add(avt[:], aps, fc1b_sb[wn][:])
        avec = small_p.tile([128, 32], BF16, tag="avec")
        nc.scalar.activation(avec[:], avt[:], AF.Sigmoid)
        nc.tensor.transpose(pst2[32 * wn:32 * (wn + 1), :], avec[:], ident[:])

    def av_gather(pst2):
        # aligned psum->sbuf copy, then DMA-flatten 32 partitions -> 1 row
        avT = small_p.tile([64, 128], BF16, tag="avT")
        nc.scalar.copy(avT[:], pst2[:])
        for wn in range(2):
            av0 = av_p.tile([1, 4096], BF16, tag=f"av0_{wn}")
            avd = dram_p.tile([4096], BF16, tag="avd")
            nc.sync.dma_start(avd[:].rearrange("(q x) -> q x", q=32),
                              avT[32 * wn:32 * (wn + 1), :])
            nc.sync.dma_start(av0[:], avd[:].unsqueeze(0))
            avvec[wn] = av0

    aexp = {}    # (wn, c) -> tile [128, 1024] layout (i, co)

    def bcast_c(wn, c):
        if True:
            ax = aexp_p.tile([128, 1024], BF16, tag=f"ax{wn}{c}")
            for ih in range(2):
                ap2 = aps2_p.tile([128, 512], F32, tag="aps2")
                for ii in range(2):
                    i = 2 * ih + ii
                    for h2 in range(2):
                        m = 4 * (2 * c + h2) + i
                        for b2 in range(2):
                            q = 2 * m + b2
                            nc.tensor.matmul(
                                ap2[64 * h2:64 * h2 + 64,
                                    256 * ii + 128 * b2:
                                    256 * ii + 128 * b2 + 128],
                                ones_sb[:],
                                avvec[wn][0:1, 128 * q:128 * (q + 1)],
                                start=True, stop=True)
                nc.scalar.copy(ax[:, 512 * ih:512 * (ih + 1)], ap2[:])
            aexp[(wn, c)] = ax

    # ---- combine: basis x coefficients -> winograd-domain weights ----
    wt = {}      # (wn, c) -> tile [128, 3072] layout (f, kw, oc)

    def combine_c(wn, c):
        if True:
            load_next_basis(4)        # prefetch the next (c)-group
            t = wt_p.tile([128, 3072], BF16, tag=f"wt{wn}{c}")
            ax = aexp[(wn, c)]

            def ab(i):
                return (ax[:, 256 * i:256 * (i + 1)].unsqueeze(1)
                        .broadcast_to([128, 3, 256]))

            def blk(ap2d, j):
                return ap2d[:, 768 * j:768 * (j + 1)].rearrange(
                    "p (kw co) -> p kw co", kw=3)

            # i-outer so only one basis tile is live per step:
            # f0 <- kh0 combine, f1-block <- halfscaled kh1 (=t), f3 <- kh2
            for i in range(4):
                bt = basis.pop((wn, c, i))
                for dst, khs in ((0, 0), (1, 1), (3, 2)):
                    d = blk(t[:], dst)
                    if i == 0:
                        nc.vector.tensor_mul(d, blk(bt[:], khs), ab(0))
                    else:
                        tmp = wtmp_p.tile([128, 768], BF16, tag="wtmp")
                        nc.vector.tensor_mul(blk(tmp[:], 0),
                                             blk(bt[:], khs), ab(i))
                        nc.vector.tensor_add(d, d, blk(tmp[:], 0))
            f1b = t[:, 768:1536]
            f2b = t[:, 1536:2304]
            # u = f0 + f3 ; f2 = 0.5u - t ; f1 = 2t + f2 = 0.5u + t
            nc.vector.tensor_add(f2b, t[:, 0:768], t[:, 2304:3072])
            nc.vector.scalar_tensor_tensor(f2b, f2b, 0.5, f1b,
                                           OP.mult, OP.subtract)
            nc.vector.scalar_tensor_tensor(f1b, f1b, 2.0, f2b,
                                           OP.mult, OP.add)
            wt[(wn, c)] = t

    # ---- V transforms ----
    v1 = {}

    def v1_transform(s):
        tiles = []
        for c in range(2):
            v = v1_p.tile([128, VFREE], BF16, tag="v1")
            vr = v[:].rearrange("p (f t w) -> p f t w", f=4, w=WP)
            xr = xpad[s][c][:].rearrange("p (h2 two w) -> p two h2 w",
                                         two=2, w=WP)
            E, O = xr[:, 0], xr[:, 1]
            nc.gpsimd.tensor_sub(vr[:, 0], E[:, 0:28], E[:, 1:29])
            nc.gpsimd.tensor_add(vr[:, 1], O[:, 0:28], E[:, 1:29])
            nc.gpsimd.tensor_sub(vr[:, 2], E[:, 1:29], O[:, 0:28])
            nc.gpsimd.tensor_sub(vr[:, 3], O[:, 0:28], O[:, 1:29])
            tiles.append(v)
        v1[s] = tiles

    v2s = {}

    def v2_transform(o1pad, c, t):
        v = v2_p.tile([128, VSFREE], BF16, tag="v2")
        vr = v[:].rearrange("p (f t w) -> p f t w", f=4, w=WP)
        xr = o1pad[c][:].rearrange("p (h2 two w) -> p two h2 w", two=2, w=WP)
        E, O = xr[:, 0], xr[:, 1]
        t4 = 4 * t
        nc.gpsimd.tensor_sub(vr[:, 0], E[:, t4:t4 + 4], E[:, t4 + 1:t4 + 5])
        nc.gpsimd.tensor_add(vr[:, 1], O[:, t4:t4 + 4], E[:, t4 + 1:t4 + 5])
        nc.gpsimd.tensor_sub(vr[:, 2], E[:, t4 + 1:t4 + 5], O[:, t4:t4 + 4])
        nc.gpsimd.tensor_sub(vr[:, 3], O[:, t4:t4 + 4], O[:, t4 + 1:t4 + 5])
        v2s[(c, t)] = v

    # ---- conv GEMM unit: 4 freqs x (3 kw x 2 ic-chunks) accumulation ----
    def conv_unit(wn, rhs, cc, t, sink, resid=False):
        pA = cps_p.tile([128, 448], F32, tag="cps")
        pB = cps_p.tile([128, 448], F32, tag="cps")
        for fi, ptile, off in ((1, pA, 0), (2, pA, 224),
                               (0, pB, 0), (3, pB, 224)):
            xtra = resid and fi in (0, 3)
            for c in range(2):
                for kw in range(3):
                    nc.tensor.matmul(
                        ptile[:, off:off + 224],
                        wt[(wn, c)][:, 768 * fi + 256 * kw + 128 * cc:
                                    768 * fi + 256 * kw + 128 * cc + 128],
                        rhs(c, fi, t, kw),
                        start=(c == 0 and kw == 0),
                        stop=(c == 1 and kw == 2 and not xtra))
            if xtra:
                # f0 += diag(1/bns) @ x_even ; f3 += diag(-1/bns) @ x_odd
                sgn = 0 if fi == 0 else 1
                xiv = rows_iv(xpad_res[cc], t)
                half = xiv[:, 0] if fi == 0 else xiv[:, 1]
                nc.tensor.matmul(ptile[:, off:off + 224], rdg_sb[cc][sgn][:],
                                 half, start=False, stop=True)
        sink(cc, t, pA, pB)

    def rhs_v1(s):
        def f(c, fi, t, kw):
            vr = v1[s][c][:].rearrange("p (f t w) -> p f t w", f=4, w=WP)
            return vr[:, fi, 4 * t:4 * t + 4, kw:kw + W]
        return f

    def rhs_v2(c, fi, t, kw):
        vr = v2s[(c, t)][:].rearrange("p (f t w) -> p f t w", f=4, w=WP)
        return vr[:, fi, :, kw:kw + W]

    # interleaved (even rows | odd rows) output view of a padded image tile
    def rows_iv(tpad, t):
        r = tpad[:].rearrange("p (h w) -> p h w", h=HP)
        return r[:, 8 * t + 1:8 * t + 9, 1:1 + W].rearrange(
            "p (u par) w -> p par u w", par=2)

    xpad_res = None

    def evac_combine(pA, pB, tmp_pool, dve_u=False):
        # pA=(m1|m2), pB=(m0|m3): E=m0+m1+m2, O=m1-m2-m3
        # each op reads at most ONE psum operand (hw restriction)
        s1 = tmp_pool.tile([128, 448], F32, tag="scmb")
        nc.scalar.copy(s1[:], pB[:])
        V = nc.vector
        V.tensor_add(s1[:], s1[:], pA[:])                  # (m1+m0 | m2+m3)
        V.tensor_add(s1[:, 0:224], s1[:, 0:224], pA[:, 224:448])
        V.tensor_sub(s1[:, 224:448], pA[:, 0:224], s1[:, 224:448])
        return s1

    def make_sink1(o1pad, pend):
        def sink1(cc, t, pA, pB):
            s1 = evac_combine(pA, pB, s1tmp_p, dve_u=False)
            pend.append((cc, t, s1))
            if len(pend) > 2:
                fcc, ft, fs = pend.pop(0)
                nc.scalar.activation(rows_iv(o1pad[fcc], ft), fs[:], AF.Relu,
                                     bias=bnb_sb[0][fcc][:],
                                     scale=bns_sb[0][fcc][:])
        return sink1

    def flush_sink1(o1pad, pend):
        while pend:
            fcc, ft, fs = pend.pop(0)
            nc.scalar.activation(rows_iv(o1pad[fcc], ft), fs[:], AF.Relu,
                                 bias=bnb_sb[0][fcc][:], scale=bns_sb[0][fcc][:])

    def make_sink2(s, pend):
        def fin2(item):
            fcc, ft, fs = item
            stg = stage_p.tile([128, 448], F32, tag="stage")
            so = stg[:].rearrange("p (u par w) -> p par u w", par=2, w=W)
            nc.scalar.activation(so, fs[:], AF.Relu,
                                 bias=bnb_sb[1][fcc][:], scale=bns_sb[1][fcc][:])
            oflat = out4[s, 128 * fcc:128 * (fcc + 1)].rearrange(
                "c h w -> c (h w)")
            nc.sync.dma_start(oflat[:, 448 * ft:448 * (ft + 1)], stg[:])

        def sink2(cc, t, pA, pB):
            s2 = evac_combine(pA, pB, s2tmp_p, dve_u=False)
            pend.append((cc, t, s2))
            if len(pend) > 2:
                fin2(pend.pop(0))
        return sink2, fin2

    # ================= prologue =================
    for c in range(2):
        load_xstage(1, c)
    load_next_basis(4)               # prime w1(0) c0 group
    padcopy_gap(0)
    pst2 = psmall_p.tile([64, 128], BF16, tag="pst")
    front_a(0, 0, pst2)
    front_a(1, 0, pst2)
    av_gather(pst2)
    bcast_c(0, 0)
    bcast_c(0, 1)
    bcast_c(1, 0)
    bcast_c(1, 1)
    v1_transform(0)
    combine_c(0, 0)                   # W1t(0)
    combine_c(0, 1)

    # ================= steady iterations =================
    for s in range(BL):
        # step 1: V1(s) already done (prologue or end of prev iter)

        # step 2: combine W2t(s), then basis DMAs for w1(s+1)
        combine_c(1, 0)               # W2t(s) from aexp(s)
        combine_c(1, 1)

        # conv1(s) GEMMs + sinks; weight-gen front for s+1 and V2 strips
        # interleave with the strips so the in-order PE queue never blocks
        o1pad = []
        for c in range(2):
            op_ = o1pad_p.tile([128, NPPAD], BF16, tag="o1pad")
            border_memset(op_)
            o1pad.append(op_)
        pend1 = []
        sink1 = make_sink1(o1pad, pend1)
        psts = {}
        for t in range(NSTR):
            for cc in range(2):
                conv_unit(0, rhs_v1(s), cc, t, sink1)
            if t >= 2:
                for c in range(2):
                    v2_transform(o1pad, c, t - 2)
            if s + 1 < BL:
                if t == 0:
                    padcopy_gap(s + 1)
                elif t == 2:
                    pst2 = psmall_p.tile([64, 128], BF16, tag="pst")
                    front_a(0, s + 1, pst2)
                    front_a(1, s + 1, pst2)
                elif t == 4:
                    av_gather(pst2)
        flush_sink1(o1pad, pend1)
        for tt in (NSTR - 2, NSTR - 1):
            for c in range(2):
                v2_transform(o1pad, c, tt)

        # step 5: c0 broadcasts for s+1 (c1 happens inside conv2 loop)
        if s + 1 < BL:
            bcast_c(0, 0)
            bcast_c(1, 0)

        # step 6: combine W1t(s+1) chunk 0 (chunk 1 inside conv2 loop)
        if s + 1 < BL:
            combine_c(0, 0)

        # step 7: conv2(s) GEMMs + sinks + stores; x staging for s+2
        if s + 2 < BL:
            for c in range(2):
                load_xstage(s + 2, c)
        pend2 = []
        sink2, fin2 = make_sink2(s, pend2)
        xpad_res = xpad[s]
        for t in range(NSTR):
            for cc in range(2):
                conv_unit(1, rhs_v2, cc, t, sink2, resid=True)
            if s + 1 < BL:
                if t == 2:
                    bcast_c(0, 1)
                    bcast_c(1, 1)
                elif t == 3:
                    combine_c(0, 1)

        while pend2:
            fin2(pend2.pop(0))

        # step 8: V1(s+1) (runs on DVE during conv2(s) GEMMs)
        if s + 1 < BL:
            v1_transform(s + 1)

    ctx.close()


_NC_CACHE = {}


def get_program():
    if "nc" not in _NC_CACHE:
        _NC_CACHE["nc"] = build_program()
    return _NC_CACHE["nc"]


def prep_inputs(inputs):
    x = np.asarray(inputs["x"], np.float32)
    f32 = lambda a: np.ascontiguousarray(np.asarray(a, np.float32))
    bf = lambda a: np.ascontiguousarray(
        np.asarray(a, np.float32).astype(ml_dtypes.bfloat16))

    # fc1 permutation: column 128*j + p holds fc1 row 16*(128*(j%2)+p) + j//2
    ns = np.array([16 * (128 * (j % 2) + p) + j // 2
                   for j in range(32) for p in range(128)])

    def pack_fc1w(w):
        return bf(np.asarray(w, np.float32)[ns, :].T)

    def pack_fc1b(b):
        return f32(np.asarray(b, np.float32)[ns].reshape(32, 128).T)

    def pack_w2(fc2_w):
        w6 = np.asarray(fc2_w, np.float32).reshape(256, 4, 64, 3, 3, 4).copy()
        w6[:, :, :, 1, :, :] *= 0.5          # pre-halve kh=1 for G-transform
        w7 = w6.transpose(5, 1, 2, 3, 4, 0)  # [i, j', ic64, kh, kw, oc]
        w8 = w7.reshape(4, 2, 2, 64, 3, 3, 256)
        w9 = w8.transpose(1, 0, 2, 3, 4, 5, 6).reshape(2, 4, 128, 2304)
        return bf(w9)

    def bn_fold(g, b, m, v):
        sc = np.asarray(g, np.float32) / np.sqrt(np.asarray(v, np.float32) + EPS)
        bia = np.asarray(b, np.float32) - np.asarray(m, np.float32) * sc
        return f32(sc.reshape(2, 128, 1)), f32(bia.reshape(2, 128, 1))

    base = {
        "rwT": f32((np.asarray(inputs["reduce_w"], np.float32).T / NPIX)
                   .reshape(2, 128, 16)),
        "rb": f32(np.asarray(inputs["reduce_b"]).reshape(16, 1)),
        "fc1wT1": pack_fc1w(inputs["w1_fc1_w"]),
        "fc1wT2": pack_fc1w(inputs["w2_fc1_w"]),
        "fc1b1": pack_fc1b(inputs["w1_fc1_b"]),
        "fc1b2": pack_fc1b(inputs["w2_fc1_b"]),
        "w2p1": pack_w2(inputs["w1_fc2_w"]),
        "w2p2": pack_w2(inputs["w2_fc2_w"]),
    }
    base["bns1"], base["bnb1"] = bn_fold(inputs["bn1_g"], inputs["bn1_b"],
                                         inputs["bn1_m"], inputs["bn1_v"])
    base["bns2"], base["bnb2"] = bn_fold(inputs["bn2_g"], inputs["bn2_b"],
                                         inputs["bn2_m"], inputs["bn2_v"])
    rb2 = (1.0 / base["bns2"]).reshape(2, 128)
    rdg = np.zeros((2, 2, 128, 128), np.float32)
    for c in range(2):
        np.fill_diagonal(rdg[c, 0], rb2[c])
        np.fill_diagonal(rdg[c, 1], -rb2[c])
    base["rdg2"] = bf(rdg)
    xb = x.astype(ml_dtypes.bfloat16)
    in_maps = []
    for i in range(NCORES):
        m = dict(base)
        m["x4b"] = np.ascontiguousarray(xb[i * BL:(i + 1) * BL])
        in_maps.append(m)
    return in_maps


def kernel(**inputs):
    in_maps = prep_inputs(inputs)
    nc = get_program()
    res = bass_utils.run_bass_kernel_spmd(nc, in_maps,
                                          core_ids=list(range(NCORES)))
    out = np.concatenate([r["out4"] for r in res.results], axis=0)
    return out.astype(np.float32)
